# revision 54
# baseline (speedup 1.0000x reference)
"""Causal self-attention with RoPE (B=2, T=2048, C=1024, H=16, D=64) on 8
Trainium2 NeuronCores.

Sharding: tensor-parallel over heads - each core owns 2 heads (QKV and output
projections sliced on the head axis); the per-core partial outputs (full
[C, B*T] each, fp16) are summed on the host in fp32.

v6 (~178us vs v3's 213.6us):
  - v transposes moved off the PE onto the DMA xbar (dma_start_transpose per
    head), deleting 32 PE transposes + 32 LDWEIGHTS + 32 DVE copies.
  - out-proj of chunk c is deferred into chunk c+1's step loop (part of the
    feed), so its y_t dependency (recip+ynorm on DVE) never blocks the PE
    queue head; the attention chunk tail is recip+ynorm only.
  - feed order E, RoPE-E, O, RoPE-O, adds, V: q/k for the next chunk are
    ready several steps before the chunk boundary (plain tile-slice operands
    everywhere: raw repeat-APs lose tile provenance and serialize against
    all PSUM writes).
  - softmax pipeline 3 deep (PV(j) three steps after S(j)); ps_st banks
    are recycled by exp, not PV, so 2 bufs suffice.
  - causal mask: one gpsimd affine_select per diag step (both heads);
    v_all ones columns via gpsimd memset (broadcast DMAs with repeated
    source lines hit a pathologically slow descriptor path).
  - PE warm-up matmuls on a DVE-memset tile ramp the DVFS p-state from
    ~5us while the first DMAs land; first-chunk x/wqkv DMAs split fine and
    alternated across the sync/scalar hwdge rings; cos/sin tables loaded
    per-chunk instead of 2MB upfront.
  - ACT queue kept almost exp-only (exp latency recycles the score PSUM
    banks): out-proj PSUM evictions go to DVE except cc 5,7; v_sb to DVE;
    output DMA batched per 2 cc (per cc, alternating rings, in the final
    tail).
"""

import sys
import types

import numpy as np

import concourse.bass as bass
import concourse.tile as tile
from concourse import bacc
from concourse import mybir
from concourse.bass_utils import run_bass_kernel_spmd

F32 = mybir.dt.float32
F16 = mybir.dt.float16

B = 2
T = 2048
C = 1024
D = 64
N_CORES = 8
BT = B * T              # 4096
TC = 512                # token chunk (free dim of most matmuls)
NQI = T // TC           # 4 qi chunks per batch
NKJ = T // 128          # 16 kj chunks per batch
KC = C // 128           # 8 contraction chunks for the projections
MASK_VAL = -240.0       # exp(0.125*(s+MASK_VAL)) underflows fp16 to 0
N_WARMUP = 10


def _install_ntff_hook():
    """bass_utils imports antenv.axon_hooks when tracing; this image lacks it.
    Recreate it from the ctypes NTFF driver so trace=True works."""
    if "antenv.axon_hooks" in sys.modules:
        return
    try:
        from trn_agent_boot.trn_boot import _ntff_profile_via_ctypes

        hook = _ntff_profile_via_ctypes("/opt/axon/libaxon_pjrt.so")
    except Exception:
        hook = None
    mod = types.ModuleType("antenv.axon_hooks")
    mod.get_axon_ntff_profile_hook = lambda: hook
    mod.set_axon_ntff_profile_hook = lambda h: None
    sys.modules["antenv.axon_hooks"] = mod


_install_ntff_hook()

X_BUFS = 3
PIPE_DEPTH = 4


def _repeat2(ap, n):
    """[128, n] AP -> [128, 2, n] AP reading the same block twice."""
    return bass.AP(tensor=ap.tensor, offset=ap.offset, ap=[ap.ap[0], [0, 2], [1, n]])


def build_nc():
    nc = bacc.Bacc(None, target_bir_lowering=False, debug=False)

    xt = nc.declare_dram_parameter("xt", [128, (BT // TC) * KC * TC], F16, isOutput=False)
    wqkv = nc.declare_dram_parameter("wqkv", [128, KC * 384], F16, isOutput=False)
    wout = nc.declare_dram_parameter("wout", [128, C], F16, isOutput=False)
    # [cos|sin], fp16
    cs = nc.declare_dram_parameter("cs", [128, 2 * T], F16, isOutput=False)
    ones = nc.declare_dram_parameter("ones", [128, 64], F16, isOutput=False)
    outT = nc.declare_dram_parameter("outT", [C, BT], F16, isOutput=True)

    with tile.TileContext(nc) as tc:
        with (
            tc.sbuf_pool(name="statics", bufs=1) as statics,
            tc.sbuf_pool(name="pool_x", bufs=X_BUFS) as pool_x,
            tc.sbuf_pool(name="pool_rope", bufs=2) as pool_rope,
            tc.sbuf_pool(name="pool_qk", bufs=2) as pool_qk,
            tc.sbuf_pool(name="pool_v", bufs=2) as pool_v,
            tc.sbuf_pool(name="pool_y", bufs=2) as pool_y,
            tc.sbuf_pool(name="pool_vs", bufs=2) as pool_vs,
            tc.sbuf_pool(name="pool_p", bufs=6) as pool_p,
            tc.sbuf_pool(name="pool_o", bufs=2) as pool_o,
            tc.sbuf_pool(name="pool_rb", bufs=2) as pool_rb,
            tc.psum_pool(name="ps_mm", bufs=2) as ps_mm,
            tc.psum_pool(name="ps_st", bufs=2) as ps_st,
            tc.psum_pool(name="ps_y", bufs=1) as ps_y,
        ):
            wqkv_a = statics.tile([128, 4 * 384], F16)
            wqkv_b = statics.tile([128, 4 * 384], F16)
            wout_sb = statics.tile([128, C], F16)
            cs_e = statics.tile([128, 2 * T], F16)   # [cos|sin]
            statics_emitted = []

            cs_loaded = set()

            def load_cs_slice(tci):
                # per-chunk cos/sin slices on the ACT hwdge queue: keeps the
                # t=0 DMA crunch small and spreads table loads over the run
                if tci in cs_loaded:
                    return
                cs_loaded.add(tci)
                csl = slice(tci * TC, (tci + 1) * TC)
                dv = cs_e.rearrange("p (two t) -> p two t", two=2)[:, :, csl]
                sv = cs.rearrange("p (two t) -> p two t", two=2)[:, :, csl]
                nc.sync.dma_start(out=dv, in_=sv)

            def emit_deferred_statics():
                if statics_emitted:
                    return
                statics_emitted.append(1)
                nc.scalar.dma_start(out=wout_sb, in_=wout[:, :])

            state = {}
            x_tiles = {}

            def x_prefetch(g):
                # steady-state only (g>=4): issue chunk g's x two chunks
                # ahead, mid-chunk, so the consuming chunk's first E matmuls
                # are ready immediately; the cold window (g<4) keeps the
                # 1-ahead pattern to avoid crowding the startup DMAs
                xt_sb = pool_x.tile([128, KC, TC], F16, tag="x", name=f"xt_{g}")
                x_tiles[g] = xt_sb
                xg = xt[:, g * KC * TC : (g + 1) * KC * TC].rearrange(
                    "p (kc n) -> p kc n", n=TC
                )
                nc.sync.dma_start(out=xt_sb[:, 0:4, :], in_=xg[:, 0:4, :])
                nc.sync.dma_start(out=xt_sb[:, 4:KC, :], in_=xg[:, 4:KC, :])

            def qkv_pieces(b, tci):
                """Generator: emits the QKV+RoPE work for chunk (b, tci) in
                small pieces so it can be interleaved into the previous
                attention chunk's step loop (keeps the PE fed while ACT runs
                exp).  Order: E, O matmuls, RoPE (so q/k are ready well
                before the chunk boundary), then V + transposes."""
                g = 4 * b + tci
                if tci == 0:
                    state[b] = (
                        pool_qk.tile([128, T], F16, tag="q", name=f"q_{b}"),
                        pool_qk.tile([128, T], F16, tag="k", name=f"k_{b}"),
                        pool_v.tile([128, 256 * NKJ], F16, tag="v", name=f"v_{b}"),
                        pool_y.tile([128, T], F16, tag="yt", name=f"y_{b}"),
                    )
                q_t, k_t, v_all, y_t = state[b]
                if g == 0:
                    # split the first chunk so the first matmul only waits
                    # for its first slices (few issues: sync-queue time is
                    # ~0.7us per dma_start)
                    xt_sb = pool_x.tile([128, KC, TC], F16, tag="x", name="xt_0")
                    xg = xt[:, 0 : KC * TC].rearrange("p (kc n) -> p kc n", n=TC)
                    # alternate the two hwdge rings; smallest critical
                    # pieces first so the first E matmuls start early
                    nc.sync.dma_start(out=wqkv_a[:, 0:384], in_=wqkv[:, 0:384])
                    nc.scalar.dma_start(out=xt_sb[:, 0, :], in_=xg[:, 0, :])
                    nc.sync.dma_start(out=wqkv_a[:, 384 : 2 * 384], in_=wqkv[:, 384 : 2 * 384])
                    nc.scalar.dma_start(out=xt_sb[:, 1, :], in_=xg[:, 1, :])
                    nc.sync.dma_start(
                        out=wqkv_a[:, 2 * 384 : 4 * 384], in_=wqkv[:, 2 * 384 : 4 * 384]
                    )
                    nc.scalar.dma_start(out=xt_sb[:, 2:4, :], in_=xg[:, 2:4, :])
                    nc.sync.dma_start(out=wqkv_b, in_=wqkv[:, 4 * 384 :])
                    nc.scalar.dma_start(out=xt_sb[:, 4:KC, :], in_=xg[:, 4:KC, :])
                    # PE warm-up on a DVE-memset zero tile: starts as soon as
                    # the DVE preamble ends (~5us), ramping the DVFS p-state
                    # while the first DMAs stream in
                    wu_src = statics.tile([128, 384], F16)
                    nc.vector.memset(wu_src, 0.0)
                    wu_ps = ps_mm.tile([128, TC], F32, tag="mm", name="wu")
                    for _ in range(N_WARMUP):
                        nc.tensor.matmul(
                            wu_ps[:, 0:384], wu_src[:, 0:128], wu_src,
                            start=True, stop=True,
                        )
                elif g in x_tiles:
                    # prefetched two chunks ahead
                    xt_sb = x_tiles.pop(g)
                else:
                    # two halves: the first E matmuls only wait ~0.5 MB
                    xt_sb = pool_x.tile(
                        [128, KC, TC], F16, tag="x", name=f"xt_{g}"
                    )
                    xg = xt[:, g * KC * TC : (g + 1) * KC * TC].rearrange(
                        "p (kc n) -> p kc n", n=TC
                    )
                    nc.sync.dma_start(out=xt_sb[:, 0:4, :], in_=xg[:, 0:4, :])
                    nc.sync.dma_start(out=xt_sb[:, 4:KC, :], in_=xg[:, 4:KC, :])

                def xsl(kc):
                    return xt_sb[:, kc, :]

                emit_deferred_statics()
                load_cs_slice(tci)
                if tci == 0:
                    # ones columns of v_all (64 per head per 256-block):
                    # gpsimd memset, not a broadcast DMA (repeated-source
                    # DMAs hit a slow descriptor path)
                    vm = v_all.rearrange("p (m c) -> p m c", c=128)
                    nc.gpsimd.memset(vm[:, :, 64:128], 1.0)
                yield
                c_sl = cs_e[:, tci * TC : (tci + 1) * TC]
                s_sl = cs_e[:, T + tci * TC : T + (tci + 1) * TC]
                pse = ps_mm.tile([128, TC], F32, tag="mm", name=f"pse_{g}")
                for kc in range(KC):
                    w_t = wqkv_a if kc < 4 else wqkv_b
                    wb = (kc % 4) * 384
                    nc.tensor.matmul(
                        pse, w_t[:, wb : wb + 128], xsl(kc),
                        start=(kc == 0), stop=(kc == KC - 1),
                    )
                    yield
                # E muls issue as soon as E stops (they overlap the O loop)
                tEC = pool_rope.tile([128, TC], F16, tag="tEC", name=f"tEC_{g}")
                nc.vector.tensor_mul(out=tEC, in0=pse, in1=c_sl)
                tES = pool_rope.tile([128, TC], F16, tag="tES", name=f"tES_{g}")
                nc.vector.tensor_mul(out=tES, in0=pse, in1=s_sl)
                yield
                pso = ps_mm.tile([128, TC], F32, tag="mm", name=f"pso_{g}")
                for kc in range(KC):
                    w_t = wqkv_a if kc < 4 else wqkv_b
                    wb = (kc % 4) * 384
                    nc.tensor.matmul(
                        pso, w_t[:, wb + 128 : wb + 256], xsl(kc),
                        start=(kc == 0), stop=(kc == KC - 1),
                    )
                    yield
                tOS = pool_rope.tile([128, TC], F16, tag="tOS", name=f"tOS_{g}")
                nc.vector.tensor_mul(out=tOS, in0=pso, in1=s_sl)
                tOC = pool_rope.tile([128, TC], F16, tag="tOC", name=f"tOC_{g}")
                nc.vector.tensor_mul(out=tOC, in0=pso, in1=c_sl)
                yield

                sl = slice(tci * TC, (tci + 1) * TC)
                # q_t rows [h0e|h0o|h1e|h1o]; E rows [q0e|q1e|k0e|k1e]
                for h in range(2):
                    he = slice(32 * h, 32 * (h + 1))
                    nc.vector.tensor_sub(
                        out=q_t[64 * h : 64 * h + 32, sl],
                        in0=tEC[he], in1=tOS[he],
                    )
                    nc.vector.tensor_add(
                        out=q_t[64 * h + 32 : 64 * h + 64, sl],
                        in0=tES[he], in1=tOC[he],
                    )
                yield
                for h in range(2):
                    ke_ = slice(64 + 32 * h, 64 + 32 * (h + 1))
                    nc.vector.tensor_sub(
                        out=k_t[64 * h : 64 * h + 32, sl],
                        in0=tEC[ke_], in1=tOS[ke_],
                    )
                    nc.vector.tensor_add(
                        out=k_t[64 * h + 32 : 64 * h + 64, sl],
                        in0=tES[ke_], in1=tOC[ke_],
                    )
                yield
                # V last: nothing needs v_all until the next chunk's diag PV
                psv = ps_mm.tile([128, TC], F32, tag="mm", name=f"psv_{g}")
                for kc in range(KC):
                    w_t = wqkv_a if kc < 4 else wqkv_b
                    wb = (kc % 4) * 384
                    nc.tensor.matmul(
                        psv,
                        w_t[:, wb + 256 : wb + 384],
                        xsl(kc),
                        start=(kc == 0),
                        stop=(kc == KC - 1),
                    )
                    yield
                # v: PSUM -> SBUF fp16, then the two per-head xbar transposes
                # into v_all (no PE involvement)
                v_sb = pool_vs.tile([128, TC], F16, tag="vs", name=f"vsb_{g}")
                nc.vector.tensor_copy(out=v_sb, in_=psv)
                va4 = v_all.rearrange("p (m h c) -> p m h c", h=2, c=128)[
                    :, 4 * tci : 4 * tci + 4
                ]
                nc.sync.dma_start_transpose(out=va4[:, :, 0, 0:64], in_=v_sb[0:64, :])
                nc.sync.dma_start_transpose(out=va4[:, :, 1, 0:64], in_=v_sb[64:128, :])
                yield

            def outproj_pieces(b, tci, tail=False):
                """Out-projection of chunk (b, tci); interleaved into the NEXT
                chunk's step loop so its y_t dependency never blocks the PE
                queue head."""
                g = 4 * b + tci
                y_t = state[b][3]
                o_big = pool_o.tile([128, KC, TC], F16, tag="o", name=f"o_{g}")
                for cc in range(KC):
                    ps = ps_mm.tile([128, TC], F32, tag="mm", name=f"op_{g}_{cc}")
                    nc.tensor.matmul(
                        ps,
                        wout_sb[:, 128 * cc : 128 * (cc + 1)],
                        y_t[:, TC * tci : TC * (tci + 1)],
                        start=True,
                        stop=True,
                    )
                    # ACT is the exp critical path: only late-chunk copies
                    # there (or alternate 4/4 in the final tail)
                    if (cc % 2 == 0) if tail else (cc in (5, 7)):
                        nc.scalar.activation(
                            out=o_big[:, cc, :], in_=ps,
                            func=mybir.ActivationFunctionType.Copy,
                        )
                    else:
                        nc.vector.tensor_copy(out=o_big[:, cc, :], in_=ps)
                    if tail:
                        eng = nc.sync if cc % 2 == 0 else nc.scalar
                        eng.dma_start(
                            out=outT[
                                128 * cc : 128 * (cc + 1), g * TC : (g + 1) * TC
                            ],
                            in_=o_big[:, cc, :],
                        )
                    elif cc % 2 == 1:
                        nc.sync.dma_start(
                            out=outT[
                                128 * (cc - 1) : 128 * (cc + 1),
                                g * TC : (g + 1) * TC,
                            ].rearrange("(cc p) n -> p cc n", p=128),
                            in_=o_big[:, cc - 1 : cc + 1, :],
                        )
                    yield

            def chain(*gens):
                for gg in gens:
                    yield from gg

            def feed_for(ci):
                """Feed pulled during attention chunk `ci`: the next chunk's
                input DMAs + a few E matmuls first (ready immediately), then
                the PREVIOUS chunk's out-proj (waits on recip+ynorm for a
                short while), then the rest of the next chunk's QKV."""
                second_last = ci + 1 == len(chunks) - 1
                if ci + 1 < len(chunks):
                    if second_last:
                        nxt = pending_gen[0] = qkv_pieces(*chunks[ci + 1])
                    else:
                        nxt = qkv_pieces(*chunks[ci + 1])
                elif pending_gen[0] is not None:
                    nxt = None
                else:
                    nxt = None
                op = outproj_pieces(*chunks[ci - 1]) if ci - 1 >= 0 else None
                leftover = pending_gen[0] if ci + 1 == len(chunks) else None

                def g():
                    if nxt is not None:
                        for _ in range(5):
                            next(nxt)
                            yield
                    if op is not None:
                        yield from op
                    if 4 <= ci + 2 < len(chunks):
                        x_prefetch(ci + 2)
                        yield
                    if nxt is not None:
                        if second_last:
                            # stop before the V group (16 more pieces of 21
                            # total; 5 already pulled); the rest feeds the
                            # last chunk so it doesn't starve
                            for _ in range(16):
                                next(nxt)
                                yield
                        else:
                            yield from nxt
                    if leftover is not None:
                        yield from leftover

                return g()

            _DONE = object()

            def pull(gen, n):
                if gen is None:
                    return
                for _ in range(n):
                    if next(gen, _DONE) is _DONE:
                        return

            def drain(gen):
                if gen is not None:
                    for _ in gen:
                        pass

            def attention_chunk(
                b, i, feed, last=False, next_opens_batch=False, n_pieces=38
            ):
                """Scores/softmax/PV for query chunk i of batch b, pulling
                feed pieces between steps."""
                q_t, k_t, v_all, y_t = state[b]
                nj = 4 * i + 4
                yacc = ps_y.tile([128, 2, TC], F32, tag="y", name=f"yacc_{b}_{i}")

                def st_of(j):
                    r = j - 4 * i
                    return 128 * r if r > 0 else 0

                n_steps = nj + PIPE_DEPTH
                # drain the feed ~2 steps early ONLY when the next chunk
                # opens a batch (its diagonal PVs read v_all immediately);
                # otherwise pace across all steps so the PE never runs dry
                # at the boundary
                margin = 6 if last else (2 if next_opens_batch else 0)
                per_step = (n_pieces + n_steps - margin - 1) // (n_steps - margin)
                pre = max(1, per_step // 2)
                post = per_step - pre

                p_tiles = {}
                for j in range(n_steps):
                    if j < nj:
                        st = st_of(j)
                        r = j - 4 * i
                        ksl = slice(128 * j, 128 * (j + 1))
                        qsl = slice(TC * i + st, TC * (i + 1))
                        ps_s = ps_st.tile(
                            [128, 2, TC], F32, tag="st", name=f"s_{b}_{i}_{j}"
                        )
                        for h in range(2):
                            hs = slice(64 * h, 64 * (h + 1))
                            nc.tensor.matmul(
                                ps_s[:, h, st:], k_t[hs, ksl], q_t[hs, qsl],
                                start=True, stop=True,
                            )
                        p_sb = pool_p.tile(
                            [128, 2, TC], F16, tag="p", name=f"p_{b}_{i}_{j}"
                        )
                        p_tiles[j] = p_sb
                        # one exp for both heads (2-bank PSUM source)
                        nc.scalar.activation(
                            out=p_sb[:, :, st:], in_=ps_s[:, :, st:],
                            func=mybir.ActivationFunctionType.Exp,
                            scale=0.125,
                        )
                        if r >= 0:
                            # causal mask on the diagonal 128-block, both
                            # heads in one gpsimd op
                            nc.gpsimd.affine_select(
                                out=p_sb[:, :, st : st + 128],
                                in_=p_sb[:, :, st : st + 128],
                                pattern=[[0, 2], [1, 128]],
                                channel_multiplier=-1,
                                base=0,
                                compare_op=mybir.AluOpType.is_ge,
                                fill=0.0,
                            )
                    pull(feed, pre)
                    if j >= PIPE_DEPTH:
                        jp = j - PIPE_DEPTH
                        st = st_of(jp)
                        pp = p_tiles.pop(jp)
                        for h in range(2):
                            nc.tensor.matmul(
                                yacc[:, h, st:],
                                v_all[
                                    :, 256 * jp + 128 * h : 256 * jp + 128 * (h + 1)
                                ],
                                pp[:, h, st:],
                                start=(jp == 0),
                                stop=(jp == nj - 1),
                            )
                    pull(feed, post)
                # recip/ynorm on the DVE queue; out-proj is deferred into the
                # next chunk's step loop, so nothing on the PE waits for this
                rb = pool_rb.tile([128, 2, TC], F32, tag="rb", name=f"rb_{b}_{i}")
                # base partition must be 0 for the custom op; rows 0:64 are
                # unused garbage recips
                nc.vector.reciprocal_approx_fast(out=rb, in_=yacc)
                for h in range(2):
                    nc.vector.tensor_mul(
                        out=y_t[64 * h : 64 * (h + 1), TC * i : (i + 1) * TC],
                        in0=yacc[0:64, h, :],
                        in1=rb[64:128, h, :],
                    )
                drain(feed)

            def outproj_tail(b, tci):
                """Last chunk's out-proj: split-contraction halves per head so
                the PE starts as soon as h0's ynorm lands; paired banks with
                alternating row groups for PE overlap."""
                g = 4 * b + tci
                y_t = state[b][3]
                sl = slice(TC * tci, TC * (tci + 1))
                o_big = pool_o.tile([128, KC, TC], F16, tag="o", name=f"o_{g}")
                for pair in range(4):
                    ccA, ccB = 2 * pair, 2 * pair + 1
                    psA = ps_mm.tile([128, TC], F32, tag="mm", name=f"opA_{pair}")
                    psB = ps_mm.tile([128, TC], F32, tag="mm", name=f"opB_{pair}")
                    for h in (0, 1):
                        hp = slice(64 * h, 64 * (h + 1))
                        nc.tensor.matmul(
                            psA, wout_sb[hp, 128 * ccA : 128 * (ccA + 1)],
                            y_t[hp, sl], start=(h == 0), stop=(h == 1),
                        )
                        nc.tensor.matmul(
                            psB, wout_sb[hp, 128 * ccB : 128 * (ccB + 1)],
                            y_t[hp, sl], start=(h == 0), stop=(h == 1),
                        )
                    nc.scalar.activation(
                        out=o_big[:, ccA, :], in_=psA,
                        func=mybir.ActivationFunctionType.Copy,
                    )
                    nc.vector.tensor_copy(out=o_big[:, ccB, :], in_=psB)
                    nc.sync.dma_start(
                        out=outT[
                            128 * ccA : 128 * (ccB + 1), g * TC : (g + 1) * TC
                        ].rearrange("(cc p) n -> p cc n", p=128),
                        in_=o_big[:, ccA : ccB + 1, :],
                    )

            chunks = [(b, i) for b in range(B) for i in range(NQI)]
            pending_gen = [None]
            gen = qkv_pieces(*chunks[0])
            drain(gen)
            for ci, (b, i) in enumerate(chunks):
                nob = ci + 1 < len(chunks) and chunks[ci + 1][1] == 0
                if ci == len(chunks) - 2:
                    np_ = 29
                elif ci == len(chunks) - 1:
                    np_ = 17
                else:
                    np_ = 38
                attention_chunk(
                    b, i, feed_for(ci), last=(ci == len(chunks) - 1),
                    next_opens_batch=nob, n_pieces=np_,
                )
            # last chunk's out-proj: the score banks are free, so use
            # them to decouple the matmuls from the copies (4 banks in
            # flight, merged 2-cc copies, DMAs on both rings)
            bl, il = chunks[-1]
            gl = 4 * bl + il
            y_tl = state[bl][3]
            o_big = pool_o.tile([128, KC, TC], F16, tag="o", name=f"o_{gl}")
            for half in range(2):
                ps4 = ps_st.tile(
                    [128, 2, TC], F32, tag="st", name=f"opt_{half}"
                )
                for q in range(2):
                    cc = 2 * half + q
                    nc.tensor.matmul(
                        ps4[:, q, :],
                        wout_sb[:, 128 * cc : 128 * (cc + 1)],
                        y_tl[:, TC * il : TC * (il + 1)],
                        start=True, stop=True,
                    )
                ps4b = ps_mm.tile([128, TC], F32, tag="mm", name=f"optb_{half}")
                cc4 = 4 + 2 * half
                nc.tensor.matmul(
                    ps4b,
                    wout_sb[:, 128 * cc4 : 128 * (cc4 + 1)],
                    y_tl[:, TC * il : TC * (il + 1)],
                    start=True, stop=True,
                )
                ps4c = ps_mm.tile([128, TC], F32, tag="mm", name=f"optc_{half}")
                nc.tensor.matmul(
                    ps4c,
                    wout_sb[:, 128 * (cc4 + 1) : 128 * (cc4 + 2)],
                    y_tl[:, TC * il : TC * (il + 1)],
                    start=True, stop=True,
                )
                # merged 2-cc copy from the score banks, singles from mm
                eng_a = nc.scalar if half == 0 else nc.vector
                if half == 0:
                    nc.scalar.activation(
                        out=o_big[:, 0:2, :], in_=ps4,
                        func=mybir.ActivationFunctionType.Copy,
                    )
                    nc.vector.tensor_copy(out=o_big[:, cc4, :], in_=ps4b)
                    nc.vector.tensor_copy(out=o_big[:, cc4 + 1, :], in_=ps4c)
                else:
                    nc.vector.tensor_copy(out=o_big[:, 2:4, :], in_=ps4)
                    nc.scalar.activation(
                        out=o_big[:, cc4, :], in_=ps4b,
                        func=mybir.ActivationFunctionType.Copy,
                    )
                    nc.scalar.activation(
                        out=o_big[:, cc4 + 1, :], in_=ps4c,
                        func=mybir.ActivationFunctionType.Copy,
                    )
                # drain this half immediately on both rings
                lo = 2 * half
                nc.sync.dma_start(
                    out=outT[
                        128 * lo : 128 * (lo + 2), gl * TC : (gl + 1) * TC
                    ].rearrange("(cc p) n -> p cc n", p=128),
                    in_=o_big[:, lo : lo + 2, :],
                )
                nc.scalar.dma_start(
                    out=outT[
                        128 * cc4 : 128 * (cc4 + 2), gl * TC : (gl + 1) * TC
                    ].rearrange("(cc p) n -> p cc n", p=128),
                    in_=o_big[:, cc4 : cc4 + 2, :],
                )

    nc.compile()
    return nc


_NC_CACHE = None


def _get_nc():
    global _NC_CACHE
    if _NC_CACHE is None:
        _NC_CACHE = build_nc()
    return _NC_CACHE


def _host_prep(x, qkv_w, out_w):
    x = np.asarray(x, dtype=np.float32)
    qkv_w = np.asarray(qkv_w, dtype=np.float32)
    out_w = np.asarray(out_w, dtype=np.float32)

    # xt[p, ((g*KC)+kc)*TC + n] = x[g*TC + n, kc*128 + p] - one contiguous
    # line per (partition, chunk) for the per-chunk DMA
    xt = np.ascontiguousarray(
        x.reshape(BT // TC, TC, KC, 128).transpose(3, 0, 2, 1).reshape(128, -1)
    ).astype(np.float16)

    # rope tables, compact: one row per frequency (broadcast to p%32 on
    # device); [cos|sin|sin|cos] fp16
    t_idx = np.arange(T, dtype=np.float64)
    inv_freq = 1.0 / (10000.0 ** (np.arange(0, D, 2, dtype=np.float64) / D))  # 32
    ang = np.outer(np.tile(inv_freq, 4), t_idx)  # [128, T]
    c_t, s_t = np.cos(ang), np.sin(ang)
    cs = np.concatenate([c_t, s_t], axis=1).astype(np.float16)  # [128, 2T]

    ones = np.ones((128, 64), np.float16)

    in_maps = []
    for core in range(N_CORES):
        h0 = 2 * core
        h1 = h0 + 1
        ev = np.arange(0, D, 2)
        od = np.arange(1, D, 2)
        e_rows = np.concatenate(
            [h0 * D + ev, h1 * D + ev, C + h0 * D + ev, C + h1 * D + ev]
        )
        o_rows = np.concatenate(
            [h0 * D + od, h1 * D + od, C + h0 * D + od, C + h1 * D + od]
        )
        v_rows = np.concatenate(
            [2 * C + h0 * D + np.arange(D), 2 * C + h1 * D + np.arange(D)]
        )
        rows = np.concatenate([e_rows, o_rows, v_rows])  # [384]
        w_part = qkv_w[rows]  # [384, C]
        # wqkv[p, kc*384 + m] = w_part[m, kc*128 + p]
        wqkv_c = np.ascontiguousarray(
            w_part.T.reshape(KC, 128, 384).transpose(1, 0, 2).reshape(128, KC * 384)
        ).astype(np.float16)
        cols = np.concatenate([h0 * D + np.arange(D), h1 * D + np.arange(D)])
        wout_c = np.ascontiguousarray(out_w[:, cols].T).astype(np.float16)  # [128, C]
        in_maps.append(
            {"xt": xt, "wqkv": wqkv_c, "wout": wout_c, "cs": cs, "ones": ones}
        )
    return in_maps


def _run(in_maps, trace=False):
    nc = _get_nc()
    return run_bass_kernel_spmd(
        nc, in_maps, core_ids=list(range(N_CORES)), trace=trace
    )


def kernel(x, qkv_w, out_w, _trace=False, _results_box=None):
    in_maps = _host_prep(x, qkv_w, out_w)
    res = _run(in_maps, trace=_trace)
    if _results_box is not None:
        _results_box.append(res)
    acc = np.zeros((C, BT), np.float32)
    for r in res.results:
        acc += r["outT"].astype(np.float32)
    out = acc.T.reshape(B, T, C)
    return np.ascontiguousarray(out)


# revision 55
# speedup vs baseline: 1.0000x; 1.0000x over previous
"""Causal self-attention with RoPE (B=2, T=2048, C=1024, H=16, D=64) on 8
Trainium2 NeuronCores.

Sharding: tensor-parallel over heads - each core owns 2 heads (QKV and output
projections sliced on the head axis); the per-core partial outputs (full
[C, B*T] each, fp16) are summed on the host in fp32.

v6 (~178us vs v3's 213.6us):
  - v transposes moved off the PE onto the DMA xbar (dma_start_transpose per
    head), deleting 32 PE transposes + 32 LDWEIGHTS + 32 DVE copies.
  - out-proj of chunk c is deferred into chunk c+1's step loop (part of the
    feed), so its y_t dependency (recip+ynorm on DVE) never blocks the PE
    queue head; the attention chunk tail is recip+ynorm only.
  - feed order E, RoPE-E, O, RoPE-O, adds, V: q/k for the next chunk are
    ready several steps before the chunk boundary (plain tile-slice operands
    everywhere: raw repeat-APs lose tile provenance and serialize against
    all PSUM writes).
  - softmax pipeline 3 deep (PV(j) three steps after S(j)); ps_st banks
    are recycled by exp, not PV, so 2 bufs suffice.
  - causal mask: one gpsimd affine_select per diag step (both heads);
    v_all ones columns via gpsimd memset (broadcast DMAs with repeated
    source lines hit a pathologically slow descriptor path).
  - PE warm-up matmuls on a DVE-memset tile ramp the DVFS p-state from
    ~5us while the first DMAs land; first-chunk x/wqkv DMAs split fine and
    alternated across the sync/scalar hwdge rings; cos/sin tables loaded
    per-chunk instead of 2MB upfront.
  - ACT queue kept almost exp-only (exp latency recycles the score PSUM
    banks): out-proj PSUM evictions go to DVE except cc 5,7; v_sb to DVE;
    output DMA batched per 2 cc (per cc, alternating rings, in the final
    tail).
"""

import sys
import types

import numpy as np

import concourse.bass as bass
import concourse.tile as tile
from concourse import bacc
from concourse import mybir
from concourse.bass_utils import run_bass_kernel_spmd

F32 = mybir.dt.float32
F16 = mybir.dt.float16

B = 2
T = 2048
C = 1024
D = 64
N_CORES = 8
BT = B * T              # 4096
TC = 512                # token chunk (free dim of most matmuls)
NQI = T // TC           # 4 qi chunks per batch
NKJ = T // 128          # 16 kj chunks per batch
KC = C // 128           # 8 contraction chunks for the projections
MASK_VAL = -240.0       # exp(0.125*(s+MASK_VAL)) underflows fp16 to 0
N_WARMUP = 10


def _install_ntff_hook():
    """bass_utils imports antenv.axon_hooks when tracing; this image lacks it.
    Recreate it from the ctypes NTFF driver so trace=True works."""
    if "antenv.axon_hooks" in sys.modules:
        return
    try:
        from trn_agent_boot.trn_boot import _ntff_profile_via_ctypes

        hook = _ntff_profile_via_ctypes("/opt/axon/libaxon_pjrt.so")
    except Exception:
        hook = None
    mod = types.ModuleType("antenv.axon_hooks")
    mod.get_axon_ntff_profile_hook = lambda: hook
    mod.set_axon_ntff_profile_hook = lambda h: None
    sys.modules["antenv.axon_hooks"] = mod


_install_ntff_hook()

X_BUFS = 3
PIPE_DEPTH = 3


def _repeat2(ap, n):
    """[128, n] AP -> [128, 2, n] AP reading the same block twice."""
    return bass.AP(tensor=ap.tensor, offset=ap.offset, ap=[ap.ap[0], [0, 2], [1, n]])


def build_nc():
    nc = bacc.Bacc(None, target_bir_lowering=False, debug=False)

    xt = nc.declare_dram_parameter("xt", [128, (BT // TC) * KC * TC], F16, isOutput=False)
    wqkv = nc.declare_dram_parameter("wqkv", [128, KC * 384], F16, isOutput=False)
    wout = nc.declare_dram_parameter("wout", [128, C], F16, isOutput=False)
    # [cos|sin], fp16
    cs = nc.declare_dram_parameter("cs", [128, 2 * T], F16, isOutput=False)
    ones = nc.declare_dram_parameter("ones", [128, 64], F16, isOutput=False)
    outT = nc.declare_dram_parameter("outT", [C, BT], F16, isOutput=True)

    with tile.TileContext(nc) as tc:
        with (
            tc.sbuf_pool(name="statics", bufs=1) as statics,
            tc.sbuf_pool(name="pool_x", bufs=X_BUFS) as pool_x,
            tc.sbuf_pool(name="pool_rope", bufs=2) as pool_rope,
            tc.sbuf_pool(name="pool_qk", bufs=2) as pool_qk,
            tc.sbuf_pool(name="pool_v", bufs=2) as pool_v,
            tc.sbuf_pool(name="pool_y", bufs=2) as pool_y,
            tc.sbuf_pool(name="pool_vs", bufs=2) as pool_vs,
            tc.sbuf_pool(name="pool_p", bufs=6) as pool_p,
            tc.sbuf_pool(name="pool_o", bufs=2) as pool_o,
            tc.sbuf_pool(name="pool_rb", bufs=2) as pool_rb,
            tc.psum_pool(name="ps_mm", bufs=2) as ps_mm,
            tc.psum_pool(name="ps_st", bufs=2) as ps_st,
            tc.psum_pool(name="ps_y", bufs=1) as ps_y,
        ):
            wqkv_a = statics.tile([128, 4 * 384], F16)
            wqkv_b = statics.tile([128, 4 * 384], F16)
            wout_sb = statics.tile([128, C], F16)
            cs_e = statics.tile([128, 2 * T], F16)   # [cos|sin]
            statics_emitted = []

            cs_loaded = set()

            def load_cs_slice(tci):
                # per-chunk cos/sin slices on the ACT hwdge queue: keeps the
                # t=0 DMA crunch small and spreads table loads over the run
                if tci in cs_loaded:
                    return
                cs_loaded.add(tci)
                csl = slice(tci * TC, (tci + 1) * TC)
                dv = cs_e.rearrange("p (two t) -> p two t", two=2)[:, :, csl]
                sv = cs.rearrange("p (two t) -> p two t", two=2)[:, :, csl]
                nc.sync.dma_start(out=dv, in_=sv)

            def emit_deferred_statics():
                if statics_emitted:
                    return
                statics_emitted.append(1)
                nc.scalar.dma_start(out=wout_sb, in_=wout[:, :])

            state = {}
            x_tiles = {}

            def x_prefetch(g):
                # steady-state only (g>=4): issue chunk g's x two chunks
                # ahead, mid-chunk, so the consuming chunk's first E matmuls
                # are ready immediately; the cold window (g<4) keeps the
                # 1-ahead pattern to avoid crowding the startup DMAs
                xt_sb = pool_x.tile([128, KC, TC], F16, tag="x", name=f"xt_{g}")
                x_tiles[g] = xt_sb
                xg = xt[:, g * KC * TC : (g + 1) * KC * TC].rearrange(
                    "p (kc n) -> p kc n", n=TC
                )
                nc.sync.dma_start(out=xt_sb[:, 0:4, :], in_=xg[:, 0:4, :])
                nc.sync.dma_start(out=xt_sb[:, 4:KC, :], in_=xg[:, 4:KC, :])

            def qkv_pieces(b, tci):
                """Generator: emits the QKV+RoPE work for chunk (b, tci) in
                small pieces so it can be interleaved into the previous
                attention chunk's step loop (keeps the PE fed while ACT runs
                exp).  Order: E, O matmuls, RoPE (so q/k are ready well
                before the chunk boundary), then V + transposes."""
                g = 4 * b + tci
                if tci == 0:
                    state[b] = (
                        pool_qk.tile([128, T], F16, tag="q", name=f"q_{b}"),
                        pool_qk.tile([128, T], F16, tag="k", name=f"k_{b}"),
                        pool_v.tile([128, 256 * NKJ], F16, tag="v", name=f"v_{b}"),
                        pool_y.tile([128, T], F16, tag="yt", name=f"y_{b}"),
                    )
                q_t, k_t, v_all, y_t = state[b]
                if g == 0:
                    # split the first chunk so the first matmul only waits
                    # for its first slices (few issues: sync-queue time is
                    # ~0.7us per dma_start)
                    xt_sb = pool_x.tile([128, KC, TC], F16, tag="x", name="xt_0")
                    xg = xt[:, 0 : KC * TC].rearrange("p (kc n) -> p kc n", n=TC)
                    # alternate the two hwdge rings; smallest critical
                    # pieces first so the first E matmuls start early
                    nc.sync.dma_start(out=wqkv_a[:, 0:384], in_=wqkv[:, 0:384])
                    nc.scalar.dma_start(out=xt_sb[:, 0, :], in_=xg[:, 0, :])
                    nc.sync.dma_start(out=wqkv_a[:, 384 : 2 * 384], in_=wqkv[:, 384 : 2 * 384])
                    nc.scalar.dma_start(out=xt_sb[:, 1, :], in_=xg[:, 1, :])
                    nc.sync.dma_start(
                        out=wqkv_a[:, 2 * 384 : 4 * 384], in_=wqkv[:, 2 * 384 : 4 * 384]
                    )
                    nc.scalar.dma_start(out=xt_sb[:, 2:4, :], in_=xg[:, 2:4, :])
                    nc.sync.dma_start(out=wqkv_b, in_=wqkv[:, 4 * 384 :])
                    nc.scalar.dma_start(out=xt_sb[:, 4:KC, :], in_=xg[:, 4:KC, :])
                    # PE warm-up on a DVE-memset zero tile: starts as soon as
                    # the DVE preamble ends (~5us), ramping the DVFS p-state
                    # while the first DMAs stream in
                    wu_src = statics.tile([128, 384], F16)
                    nc.vector.memset(wu_src, 0.0)
                    wu_ps = ps_mm.tile([128, TC], F32, tag="mm", name="wu")
                    for _ in range(N_WARMUP):
                        nc.tensor.matmul(
                            wu_ps[:, 0:384], wu_src[:, 0:128], wu_src,
                            start=True, stop=True,
                        )
                elif g in x_tiles:
                    # prefetched two chunks ahead
                    xt_sb = x_tiles.pop(g)
                else:
                    # two halves: the first E matmuls only wait ~0.5 MB
                    xt_sb = pool_x.tile(
                        [128, KC, TC], F16, tag="x", name=f"xt_{g}"
                    )
                    xg = xt[:, g * KC * TC : (g + 1) * KC * TC].rearrange(
                        "p (kc n) -> p kc n", n=TC
                    )
                    nc.sync.dma_start(out=xt_sb[:, 0:4, :], in_=xg[:, 0:4, :])
                    nc.sync.dma_start(out=xt_sb[:, 4:KC, :], in_=xg[:, 4:KC, :])

                def xsl(kc):
                    return xt_sb[:, kc, :]

                emit_deferred_statics()
                load_cs_slice(tci)
                if tci == 0:
                    # ones columns of v_all (64 per head per 256-block):
                    # gpsimd memset, not a broadcast DMA (repeated-source
                    # DMAs hit a slow descriptor path)
                    vm = v_all.rearrange("p (m c) -> p m c", c=128)
                    nc.gpsimd.memset(vm[:, :, 64:128], 1.0)
                yield
                c_sl = cs_e[:, tci * TC : (tci + 1) * TC]
                s_sl = cs_e[:, T + tci * TC : T + (tci + 1) * TC]
                pse = ps_mm.tile([128, TC], F32, tag="mm", name=f"pse_{g}")
                for kc in range(KC):
                    w_t = wqkv_a if kc < 4 else wqkv_b
                    wb = (kc % 4) * 384
                    nc.tensor.matmul(
                        pse, w_t[:, wb : wb + 128], xsl(kc),
                        start=(kc == 0), stop=(kc == KC - 1),
                    )
                    yield
                # E muls issue as soon as E stops (they overlap the O loop)
                tEC = pool_rope.tile([128, TC], F16, tag="tEC", name=f"tEC_{g}")
                nc.vector.tensor_mul(out=tEC, in0=pse, in1=c_sl)
                tES = pool_rope.tile([128, TC], F16, tag="tES", name=f"tES_{g}")
                nc.vector.tensor_mul(out=tES, in0=pse, in1=s_sl)
                yield
                pso = ps_mm.tile([128, TC], F32, tag="mm", name=f"pso_{g}")
                for kc in range(KC):
                    w_t = wqkv_a if kc < 4 else wqkv_b
                    wb = (kc % 4) * 384
                    nc.tensor.matmul(
                        pso, w_t[:, wb + 128 : wb + 256], xsl(kc),
                        start=(kc == 0), stop=(kc == KC - 1),
                    )
                    yield
                tOS = pool_rope.tile([128, TC], F16, tag="tOS", name=f"tOS_{g}")
                nc.vector.tensor_mul(out=tOS, in0=pso, in1=s_sl)
                tOC = pool_rope.tile([128, TC], F16, tag="tOC", name=f"tOC_{g}")
                nc.vector.tensor_mul(out=tOC, in0=pso, in1=c_sl)
                yield

                sl = slice(tci * TC, (tci + 1) * TC)
                # q_t rows [h0e|h0o|h1e|h1o]; E rows [q0e|q1e|k0e|k1e]
                for h in range(2):
                    he = slice(32 * h, 32 * (h + 1))
                    nc.vector.tensor_sub(
                        out=q_t[64 * h : 64 * h + 32, sl],
                        in0=tEC[he], in1=tOS[he],
                    )
                    nc.vector.tensor_add(
                        out=q_t[64 * h + 32 : 64 * h + 64, sl],
                        in0=tES[he], in1=tOC[he],
                    )
                yield
                for h in range(2):
                    ke_ = slice(64 + 32 * h, 64 + 32 * (h + 1))
                    nc.vector.tensor_sub(
                        out=k_t[64 * h : 64 * h + 32, sl],
                        in0=tEC[ke_], in1=tOS[ke_],
                    )
                    nc.vector.tensor_add(
                        out=k_t[64 * h + 32 : 64 * h + 64, sl],
                        in0=tES[ke_], in1=tOC[ke_],
                    )
                yield
                # V last: nothing needs v_all until the next chunk's diag PV
                psv = ps_mm.tile([128, TC], F32, tag="mm", name=f"psv_{g}")
                for kc in range(KC):
                    w_t = wqkv_a if kc < 4 else wqkv_b
                    wb = (kc % 4) * 384
                    nc.tensor.matmul(
                        psv,
                        w_t[:, wb + 256 : wb + 384],
                        xsl(kc),
                        start=(kc == 0),
                        stop=(kc == KC - 1),
                    )
                    yield
                # v: PSUM -> SBUF fp16, then the two per-head xbar transposes
                # into v_all (no PE involvement)
                v_sb = pool_vs.tile([128, TC], F16, tag="vs", name=f"vsb_{g}")
                nc.vector.tensor_copy(out=v_sb, in_=psv)
                va4 = v_all.rearrange("p (m h c) -> p m h c", h=2, c=128)[
                    :, 4 * tci : 4 * tci + 4
                ]
                nc.sync.dma_start_transpose(out=va4[:, :, 0, 0:64], in_=v_sb[0:64, :])
                nc.sync.dma_start_transpose(out=va4[:, :, 1, 0:64], in_=v_sb[64:128, :])
                yield

            def outproj_pieces(b, tci, tail=False):
                """Out-projection of chunk (b, tci); interleaved into the NEXT
                chunk's step loop so its y_t dependency never blocks the PE
                queue head."""
                g = 4 * b + tci
                y_t = state[b][3]
                o_big = pool_o.tile([128, KC, TC], F16, tag="o", name=f"o_{g}")
                for cc in range(KC):
                    ps = ps_mm.tile([128, TC], F32, tag="mm", name=f"op_{g}_{cc}")
                    nc.tensor.matmul(
                        ps,
                        wout_sb[:, 128 * cc : 128 * (cc + 1)],
                        y_t[:, TC * tci : TC * (tci + 1)],
                        start=True,
                        stop=True,
                    )
                    # ACT is the exp critical path: only late-chunk copies
                    # there (or alternate 4/4 in the final tail)
                    if (cc % 2 == 0) if tail else (cc in (5, 7)):
                        nc.scalar.activation(
                            out=o_big[:, cc, :], in_=ps,
                            func=mybir.ActivationFunctionType.Copy,
                        )
                    else:
                        nc.vector.tensor_copy(out=o_big[:, cc, :], in_=ps)
                    if tail:
                        eng = nc.sync if cc % 2 == 0 else nc.scalar
                        eng.dma_start(
                            out=outT[
                                128 * cc : 128 * (cc + 1), g * TC : (g + 1) * TC
                            ],
                            in_=o_big[:, cc, :],
                        )
                    elif cc % 2 == 1:
                        nc.sync.dma_start(
                            out=outT[
                                128 * (cc - 1) : 128 * (cc + 1),
                                g * TC : (g + 1) * TC,
                            ].rearrange("(cc p) n -> p cc n", p=128),
                            in_=o_big[:, cc - 1 : cc + 1, :],
                        )
                    yield

            def chain(*gens):
                for gg in gens:
                    yield from gg

            def feed_for(ci):
                """Feed pulled during attention chunk `ci`: the next chunk's
                input DMAs + a few E matmuls first (ready immediately), then
                the PREVIOUS chunk's out-proj (waits on recip+ynorm for a
                short while), then the rest of the next chunk's QKV."""
                second_last = ci + 1 == len(chunks) - 1
                if ci + 1 < len(chunks):
                    if second_last:
                        nxt = pending_gen[0] = qkv_pieces(*chunks[ci + 1])
                    else:
                        nxt = qkv_pieces(*chunks[ci + 1])
                elif pending_gen[0] is not None:
                    nxt = None
                else:
                    nxt = None
                op = outproj_pieces(*chunks[ci - 1]) if ci - 1 >= 0 else None
                leftover = pending_gen[0] if ci + 1 == len(chunks) else None

                def g():
                    if nxt is not None:
                        for _ in range(5):
                            next(nxt)
                            yield
                    if op is not None:
                        yield from op
                    if 4 <= ci + 2 < len(chunks):
                        x_prefetch(ci + 2)
                        yield
                    if nxt is not None:
                        if second_last:
                            # stop before the V group (16 more pieces of 21
                            # total; 5 already pulled); the rest feeds the
                            # last chunk so it doesn't starve
                            for _ in range(16):
                                next(nxt)
                                yield
                        else:
                            yield from nxt
                    if leftover is not None:
                        yield from leftover

                return g()

            _DONE = object()

            def pull(gen, n):
                if gen is None:
                    return
                for _ in range(n):
                    if next(gen, _DONE) is _DONE:
                        return

            def drain(gen):
                if gen is not None:
                    for _ in gen:
                        pass

            def attention_chunk(
                b, i, feed, last=False, next_opens_batch=False, n_pieces=38
            ):
                """Scores/softmax/PV for query chunk i of batch b, pulling
                feed pieces between steps."""
                q_t, k_t, v_all, y_t = state[b]
                nj = 4 * i + 4
                yacc = ps_y.tile([128, 2, TC], F32, tag="y", name=f"yacc_{b}_{i}")

                def st_of(j):
                    r = j - 4 * i
                    return 128 * r if r > 0 else 0

                n_steps = nj + PIPE_DEPTH
                # drain the feed ~2 steps early ONLY when the next chunk
                # opens a batch (its diagonal PVs read v_all immediately);
                # otherwise pace across all steps so the PE never runs dry
                # at the boundary
                margin = 6 if last else (2 if next_opens_batch else 0)
                per_step = (n_pieces + n_steps - margin - 1) // (n_steps - margin)
                pre = max(1, per_step // 2)
                post = per_step - pre

                p_tiles = {}
                for j in range(n_steps):
                    if j < nj:
                        st = st_of(j)
                        r = j - 4 * i
                        ksl = slice(128 * j, 128 * (j + 1))
                        qsl = slice(TC * i + st, TC * (i + 1))
                        ps_s = ps_st.tile(
                            [128, 2, TC], F32, tag="st", name=f"s_{b}_{i}_{j}"
                        )
                        for h in range(2):
                            hs = slice(64 * h, 64 * (h + 1))
                            nc.tensor.matmul(
                                ps_s[:, h, st:], k_t[hs, ksl], q_t[hs, qsl],
                                start=True, stop=True,
                            )
                        p_sb = pool_p.tile(
                            [128, 2, TC], F16, tag="p", name=f"p_{b}_{i}_{j}"
                        )
                        p_tiles[j] = p_sb
                        # one exp for both heads (2-bank PSUM source)
                        nc.scalar.activation(
                            out=p_sb[:, :, st:], in_=ps_s[:, :, st:],
                            func=mybir.ActivationFunctionType.Exp,
                            scale=0.125,
                        )
                        if r >= 0:
                            # causal mask on the diagonal 128-block, both
                            # heads in one gpsimd op
                            nc.gpsimd.affine_select(
                                out=p_sb[:, :, st : st + 128],
                                in_=p_sb[:, :, st : st + 128],
                                pattern=[[0, 2], [1, 128]],
                                channel_multiplier=-1,
                                base=0,
                                compare_op=mybir.AluOpType.is_ge,
                                fill=0.0,
                            )
                    pull(feed, pre)
                    if j >= PIPE_DEPTH:
                        jp = j - PIPE_DEPTH
                        st = st_of(jp)
                        pp = p_tiles.pop(jp)
                        for h in range(2):
                            nc.tensor.matmul(
                                yacc[:, h, st:],
                                v_all[
                                    :, 256 * jp + 128 * h : 256 * jp + 128 * (h + 1)
                                ],
                                pp[:, h, st:],
                                start=(jp == 0),
                                stop=(jp == nj - 1),
                            )
                    pull(feed, post)
                # recip/ynorm on the DVE queue; out-proj is deferred into the
                # next chunk's step loop, so nothing on the PE waits for this
                rb = pool_rb.tile([128, 2, TC], F32, tag="rb", name=f"rb_{b}_{i}")
                # base partition must be 0 for the custom op; rows 0:64 are
                # unused garbage recips
                nc.vector.reciprocal_approx_fast(out=rb, in_=yacc)
                for h in range(2):
                    nc.vector.tensor_mul(
                        out=y_t[64 * h : 64 * (h + 1), TC * i : (i + 1) * TC],
                        in0=yacc[0:64, h, :],
                        in1=rb[64:128, h, :],
                    )
                drain(feed)

            def outproj_tail(b, tci):
                """Last chunk's out-proj: split-contraction halves per head so
                the PE starts as soon as h0's ynorm lands; paired banks with
                alternating row groups for PE overlap."""
                g = 4 * b + tci
                y_t = state[b][3]
                sl = slice(TC * tci, TC * (tci + 1))
                o_big = pool_o.tile([128, KC, TC], F16, tag="o", name=f"o_{g}")
                for pair in range(4):
                    ccA, ccB = 2 * pair, 2 * pair + 1
                    psA = ps_mm.tile([128, TC], F32, tag="mm", name=f"opA_{pair}")
                    psB = ps_mm.tile([128, TC], F32, tag="mm", name=f"opB_{pair}")
                    for h in (0, 1):
                        hp = slice(64 * h, 64 * (h + 1))
                        nc.tensor.matmul(
                            psA, wout_sb[hp, 128 * ccA : 128 * (ccA + 1)],
                            y_t[hp, sl], start=(h == 0), stop=(h == 1),
                        )
                        nc.tensor.matmul(
                            psB, wout_sb[hp, 128 * ccB : 128 * (ccB + 1)],
                            y_t[hp, sl], start=(h == 0), stop=(h == 1),
                        )
                    nc.scalar.activation(
                        out=o_big[:, ccA, :], in_=psA,
                        func=mybir.ActivationFunctionType.Copy,
                    )
                    nc.vector.tensor_copy(out=o_big[:, ccB, :], in_=psB)
                    nc.sync.dma_start(
                        out=outT[
                            128 * ccA : 128 * (ccB + 1), g * TC : (g + 1) * TC
                        ].rearrange("(cc p) n -> p cc n", p=128),
                        in_=o_big[:, ccA : ccB + 1, :],
                    )

            chunks = [(b, i) for b in range(B) for i in range(NQI)]
            pending_gen = [None]
            gen = qkv_pieces(*chunks[0])
            drain(gen)
            for ci, (b, i) in enumerate(chunks):
                nob = ci + 1 < len(chunks) and chunks[ci + 1][1] == 0
                if ci == len(chunks) - 2:
                    np_ = 29
                elif ci == len(chunks) - 1:
                    np_ = 17
                else:
                    np_ = 38
                attention_chunk(
                    b, i, feed_for(ci), last=(ci == len(chunks) - 1),
                    next_opens_batch=nob, n_pieces=np_,
                )
            # last chunk's out-proj: the score banks are free, so use
            # them to decouple the matmuls from the copies (4 banks in
            # flight, merged 2-cc copies, DMAs on both rings)
            bl, il = chunks[-1]
            gl = 4 * bl + il
            y_tl = state[bl][3]
            o_big = pool_o.tile([128, KC, TC], F16, tag="o", name=f"o_{gl}")
            for half in range(2):
                ps4 = ps_st.tile(
                    [128, 2, TC], F32, tag="st", name=f"opt_{half}"
                )
                for q in range(2):
                    cc = 2 * half + q
                    nc.tensor.matmul(
                        ps4[:, q, :],
                        wout_sb[:, 128 * cc : 128 * (cc + 1)],
                        y_tl[:, TC * il : TC * (il + 1)],
                        start=True, stop=True,
                    )
                ps4b = ps_mm.tile([128, TC], F32, tag="mm", name=f"optb_{half}")
                cc4 = 4 + 2 * half
                nc.tensor.matmul(
                    ps4b,
                    wout_sb[:, 128 * cc4 : 128 * (cc4 + 1)],
                    y_tl[:, TC * il : TC * (il + 1)],
                    start=True, stop=True,
                )
                ps4c = ps_mm.tile([128, TC], F32, tag="mm", name=f"optc_{half}")
                nc.tensor.matmul(
                    ps4c,
                    wout_sb[:, 128 * (cc4 + 1) : 128 * (cc4 + 2)],
                    y_tl[:, TC * il : TC * (il + 1)],
                    start=True, stop=True,
                )
                # merged 2-cc copy from the score banks, singles from mm
                eng_a = nc.scalar if half == 0 else nc.vector
                if half == 0:
                    nc.scalar.activation(
                        out=o_big[:, 0:2, :], in_=ps4,
                        func=mybir.ActivationFunctionType.Copy,
                    )
                    nc.vector.tensor_copy(out=o_big[:, cc4, :], in_=ps4b)
                    nc.vector.tensor_copy(out=o_big[:, cc4 + 1, :], in_=ps4c)
                else:
                    nc.vector.tensor_copy(out=o_big[:, 2:4, :], in_=ps4)
                    nc.scalar.activation(
                        out=o_big[:, cc4, :], in_=ps4b,
                        func=mybir.ActivationFunctionType.Copy,
                    )
                    nc.scalar.activation(
                        out=o_big[:, cc4 + 1, :], in_=ps4c,
                        func=mybir.ActivationFunctionType.Copy,
                    )
                # drain this half immediately on both rings
                lo = 2 * half
                nc.sync.dma_start(
                    out=outT[
                        128 * lo : 128 * (lo + 2), gl * TC : (gl + 1) * TC
                    ].rearrange("(cc p) n -> p cc n", p=128),
                    in_=o_big[:, lo : lo + 2, :],
                )
                nc.scalar.dma_start(
                    out=outT[
                        128 * cc4 : 128 * (cc4 + 2), gl * TC : (gl + 1) * TC
                    ].rearrange("(cc p) n -> p cc n", p=128),
                    in_=o_big[:, cc4 : cc4 + 2, :],
                )

    nc.compile()
    return nc


_NC_CACHE = None


def _get_nc():
    global _NC_CACHE
    if _NC_CACHE is None:
        _NC_CACHE = build_nc()
    return _NC_CACHE


def _host_prep(x, qkv_w, out_w):
    x = np.asarray(x, dtype=np.float32)
    qkv_w = np.asarray(qkv_w, dtype=np.float32)
    out_w = np.asarray(out_w, dtype=np.float32)

    # xt[p, ((g*KC)+kc)*TC + n] = x[g*TC + n, kc*128 + p] - one contiguous
    # line per (partition, chunk) for the per-chunk DMA
    xt = np.ascontiguousarray(
        x.reshape(BT // TC, TC, KC, 128).transpose(3, 0, 2, 1).reshape(128, -1)
    ).astype(np.float16)

    # rope tables, compact: one row per frequency (broadcast to p%32 on
    # device); [cos|sin|sin|cos] fp16
    t_idx = np.arange(T, dtype=np.float64)
    inv_freq = 1.0 / (10000.0 ** (np.arange(0, D, 2, dtype=np.float64) / D))  # 32
    ang = np.outer(np.tile(inv_freq, 4), t_idx)  # [128, T]
    c_t, s_t = np.cos(ang), np.sin(ang)
    cs = np.concatenate([c_t, s_t], axis=1).astype(np.float16)  # [128, 2T]

    ones = np.ones((128, 64), np.float16)

    in_maps = []
    for core in range(N_CORES):
        h0 = 2 * core
        h1 = h0 + 1
        ev = np.arange(0, D, 2)
        od = np.arange(1, D, 2)
        e_rows = np.concatenate(
            [h0 * D + ev, h1 * D + ev, C + h0 * D + ev, C + h1 * D + ev]
        )
        o_rows = np.concatenate(
            [h0 * D + od, h1 * D + od, C + h0 * D + od, C + h1 * D + od]
        )
        v_rows = np.concatenate(
            [2 * C + h0 * D + np.arange(D), 2 * C + h1 * D + np.arange(D)]
        )
        rows = np.concatenate([e_rows, o_rows, v_rows])  # [384]
        w_part = qkv_w[rows]  # [384, C]
        # wqkv[p, kc*384 + m] = w_part[m, kc*128 + p]
        wqkv_c = np.ascontiguousarray(
            w_part.T.reshape(KC, 128, 384).transpose(1, 0, 2).reshape(128, KC * 384)
        ).astype(np.float16)
        cols = np.concatenate([h0 * D + np.arange(D), h1 * D + np.arange(D)])
        wout_c = np.ascontiguousarray(out_w[:, cols].T).astype(np.float16)  # [128, C]
        in_maps.append(
            {"xt": xt, "wqkv": wqkv_c, "wout": wout_c, "cs": cs, "ones": ones}
        )
    return in_maps


def _run(in_maps, trace=False):
    nc = _get_nc()
    return run_bass_kernel_spmd(
        nc, in_maps, core_ids=list(range(N_CORES)), trace=trace
    )


def kernel(x, qkv_w, out_w, _trace=False, _results_box=None):
    in_maps = _host_prep(x, qkv_w, out_w)
    res = _run(in_maps, trace=_trace)
    if _results_box is not None:
        _results_box.append(res)
    acc = np.zeros((C, BT), np.float32)
    for r in res.results:
        acc += r["outT"].astype(np.float32)
    out = acc.T.reshape(B, T, C)
    return np.ascontiguousarray(out)
